# revision 3
# baseline (speedup 1.0000x reference)
"""Grouped-experts MLP (MoE) kernel for Trainium2, expert-parallel over 8 cores.

Problem: x[B=2, E=8, N=1024, D=1024]; per expert e:
    out[:, e] = GELU(x[:, e] @ w1[e] + b1[e]) @ w2[e] + b2[e]
with w1[e]: [D=1024, H=4096], w2[e]: [H=4096, D=1024].

Sharding: expert axis across the 8 NeuronCores (core e owns expert e).
The host performs the "all-to-all": it hands core e the slab x[:, e]
(pre-transposed to [D, T] so the contraction dim lands on SBUF partitions)
plus expert e's weights, and reassembles the full output afterward.

Per-core kernel (T = B*N = 2048 tokens):
  layer 1 computes hT[H, T] = w1.T @ xT in H-chunks of 512, GELU fused into
  the PSUM->SBUF eviction on the scalar engine (bias b1 is per-partition).
  layer 2 accumulates out[T, D] += hT_chunk.T-slices @ w2_chunk into an
  SBUF-resident accumulator via DVE adds; b2 is added with a K=1 matmul
  folded into the first chunk's PSUM group.
All matmuls run as float32r (full fp32 data, 1 cycle/row at free-dim 512).
"""

import numpy as np

import concourse.bacc as bacc
import concourse.mybir as mybir
import concourse.tile as tile
from concourse.bass_utils import run_bass_kernel_spmd

B, E, N, D, H = 2, 8, 1024, 1024, 4096
T = B * N          # tokens per expert
P = 128
N_CORES = 8

T_HALF = 1024      # token half processed per outer iteration
H_CHUNK = 512      # H processed per inner chunk
N_TH = T // T_HALF           # 2
N_HC = H // H_CHUNK          # 8
KD = D // P                  # 8 k-tiles over D
HS = H_CHUNK // P            # 4 h-subtiles per chunk
TS = T_HALF // P             # 8 token subtiles per half
DC = D // 512                # 2 output column chunks

F32 = mybir.dt.float32
F32R = mybir.dt.float32r
GELU = mybir.ActivationFunctionType.Gelu


def build_nc():
    nc = bacc.Bacc("TRN2", target_bir_lowering=False, debug=False)

    xT = nc.dram_tensor("xT", [D, T], F32, kind="ExternalInput")
    w1 = nc.dram_tensor("w1", [D, H], F32, kind="ExternalInput")
    b1 = nc.dram_tensor("b1", [P, H // P], F32, kind="ExternalInput")
    w2 = nc.dram_tensor("w2", [H, D], F32, kind="ExternalInput")
    b2 = nc.dram_tensor("b2", [1, D], F32, kind="ExternalInput")
    onesd = nc.dram_tensor("ones", [1, P], F32, kind="ExternalInput")
    out = nc.dram_tensor("out", [T, D], F32, kind="ExternalOutput")

    with tile.TileContext(nc) as tc:
        with (
            tc.tile_pool(name="const", bufs=1) as constp,
            tc.tile_pool(name="xTp", bufs=1) as xTp,
            tc.tile_pool(name="outp", bufs=1) as outp,
            tc.tile_pool(name="w1p", bufs=12) as w1p,
            tc.tile_pool(name="w2p", bufs=12) as w2p,
            tc.tile_pool(name="hTp", bufs=16) as hTp,
            tc.tile_pool(name="ps1p", bufs=3, space="PSUM") as ps1p,
            tc.tile_pool(name="ps2p", bufs=3, space="PSUM") as ps2p,
        ):
            b1sb = constp.tile([P, H // P], F32, name="b1sb")
            nc.sync.dma_start(b1sb[:], b1[:])
            b2sb = constp.tile([1, D], F32R, name="b2sb")
            nc.sync.dma_start(b2sb[:], b2[:].bitcast(F32R))
            ones = constp.tile([1, P], F32R, name="ones")
            nc.sync.dma_start(ones[:], onesd[:].bitcast(F32R))

            # xT resident: 8 tiles [128, 2048]
            xTs = []
            for k in range(KD):
                t = xTp.tile([P, T], F32R, name=f"xT{k}", tag=f"xT{k}")
                nc.sync.dma_start(t[:], xT[k * P:(k + 1) * P, :].bitcast(F32R))
                xTs.append(t)

            for th in range(N_TH):
                t0 = th * T_HALF
                outs = []
                for ts in range(TS):
                    t = outp.tile([P, D], F32, name=f"out{ts}", tag=f"out{ts}")
                    outs.append(t)

                for hc in range(N_HC):
                    h0 = hc * H_CHUNK
                    # stream w1 chunk: [128, 512] per k-tile
                    w1t = []
                    for k in range(KD):
                        t = w1p.tile([P, H_CHUNK], F32R, name="w1t", tag="w1t")
                        nc.sync.dma_start(
                            t[:], w1[k * P:(k + 1) * P, h0:h0 + H_CHUNK].bitcast(F32R))
                        w1t.append(t)

                    # layer 1: hT chunk [512, 1024] as 8 tiles [128, 512]
                    hTt = {}
                    for hs in range(HS):
                        for tq in range(T_HALF // 512):
                            p = ps1p.tile([P, 512], F32, name="ps1", tag="ps1")
                            for k in range(KD):
                                nc.tensor.matmul(
                                    p[:],
                                    w1t[k][:, hs * P:(hs + 1) * P],
                                    xTs[k][:, t0 + tq * 512: t0 + (tq + 1) * 512],
                                    start=(k == 0),
                                    stop=(k == KD - 1),
                                )
                            ht = hTp.tile([P, 512], F32R, name="hTt", tag="hTt")
                            nc.scalar.activation(
                                ht[:], p[:], GELU,
                                bias=b1sb[:, hc * HS + hs: hc * HS + hs + 1])
                            hTt[(hs, tq)] = ht

                    # stream w2 chunk: [128, 512] per (h-subtile, d-chunk)
                    w2t = {}
                    for hs in range(HS):
                        for dc in range(DC):
                            t = w2p.tile([P, 512], F32R, name="w2t", tag="w2t")
                            nc.sync.dma_start(
                                t[:],
                                w2[h0 + hs * P: h0 + (hs + 1) * P,
                                   dc * 512:(dc + 1) * 512].bitcast(F32R))
                            w2t[(hs, dc)] = t

                    # layer 2: partial out for this H chunk
                    for ts in range(TS):
                        tq, to = divmod(ts * P, 512)
                        for dc in range(DC):
                            p = ps2p.tile([P, 512], F32, name="ps2", tag="ps2")
                            for hs in range(HS):
                                nc.tensor.matmul(
                                    p[:],
                                    hTt[(hs, tq)][:, to:to + P],
                                    w2t[(hs, dc)][:],
                                    start=(hs == 0),
                                    stop=(hs == HS - 1 and hc != 0),
                                )
                            if hc == 0:
                                # fold b2 into the first chunk's PSUM group
                                nc.tensor.matmul(
                                    p[:],
                                    ones[:],
                                    b2sb[0:1, dc * 512:(dc + 1) * 512],
                                    start=False,
                                    stop=True,
                                )
                                nc.vector.tensor_copy(
                                    outs[ts][:, dc * 512:(dc + 1) * 512], p[:])
                            else:
                                nc.vector.tensor_add(
                                    outs[ts][:, dc * 512:(dc + 1) * 512],
                                    outs[ts][:, dc * 512:(dc + 1) * 512],
                                    p[:])

                for ts in range(TS):
                    nc.sync.dma_start(
                        out[t0 + ts * P: t0 + (ts + 1) * P, :], outs[ts][:])

    nc.compile()
    return nc


def make_in_map(x_e, w1_e, b1_e, w2_e, b2_e):
    """Per-core input map from one expert's full-precision slabs."""
    xT = np.ascontiguousarray(x_e.reshape(T, D).T)
    return {
        "xT": xT,
        "w1": np.ascontiguousarray(w1_e),
        "b1": np.ascontiguousarray(b1_e.reshape(H // P, P).T),
        "w2": np.ascontiguousarray(w2_e),
        "b2": np.ascontiguousarray(b2_e.reshape(1, D)),
        "ones": np.ones((1, P), np.float32),
    }


_NC_CACHE = None


def _get_nc():
    global _NC_CACHE
    if _NC_CACHE is None:
        _NC_CACHE = build_nc()
    return _NC_CACHE


def kernel(x, w1, b1, w2, b2, trace=False):
    x = np.asarray(x, dtype=np.float32)
    w1 = np.asarray(w1, dtype=np.float32)
    b1 = np.asarray(b1, dtype=np.float32)
    w2 = np.asarray(w2, dtype=np.float32)
    b2 = np.asarray(b2, dtype=np.float32)

    nc = _get_nc()
    in_maps = [
        make_in_map(x[:, e], w1[e], b1[e], w2[e], b2[e]) for e in range(N_CORES)
    ]
    res = run_bass_kernel_spmd(
        nc, in_maps, core_ids=list(range(N_CORES)), trace=trace)
    out = np.empty((B, E, N, D), np.float32)
    for e in range(N_CORES):
        out[:, e] = res.results[e]["out"].reshape(B, N, D)
    if trace:
        return out, res
    return out
